# revision 1
# baseline (speedup 1.0000x reference)
"""Trainium2 Bass kernel for nn_AttentionEncoder (B=32, S=2048, H=1024).

Strategy: pure data-parallel over batch across 8 NeuronCores (4 batches
per core). All heavy math runs on-device:

  scores[k, s]  = sum_h W_en[k,h] * enhy[b,s,h]        (PE, bf16 -> fp32 PSUM)
  agg           = tanh(scores + dehy@W_de.T + b_en)    (ACT, bias fused)
  warp[s]       = sum_k agg[k,s] * W_warp[k]           (PE)
  ee            = exp(warp)                            (ACT)
  w             = ee / past_attn                       (DVE reciprocal+mul)
  attn          = w / sum(w)                           (deferred normalize)
  c_enc[h]      = (sum_s w[s]*enhy[b,s,h]) / sum(w)    (PE, deferred normalize)

The host pre-packs per-core shards: enhy in both [h, b*s] (for the score
matmul's moving operand) and [b*s, h] (for the context matmul) layouts,
cast to bf16, plus transposed weights. Compiled NEFF is cached per process.
"""

import numpy as np
import ml_dtypes

B, S, H = 32, 2048, 1024
NCORES = 8
BL = B // NCORES          # batches per core
P = 128                   # partition width
KT = H // P               # 8 k-tiles
HT = H // P               # 8 h-tiles
CHUNK = 512               # score-chunk columns (PSUM bank = 512 fp32)
NCHUNK = S // CHUNK       # 4 chunks per batch
ST = CHUNK // P           # 4 s-tiles per chunk

_COMPILED = {}


def _build_nc():
    import concourse.bass as bass
    import concourse.mybir as mybir
    import concourse.tile as tile
    from concourse import bacc
    from contextlib import ExitStack

    dt = mybir.dt
    BF = dt.bfloat16
    F32 = dt.float32
    AF = mybir.ActivationFunctionType
    AX = mybir.AxisListType

    nc = bacc.Bacc("TRN2", target_bir_lowering=False, debug=False,
                   num_devices=NCORES)

    enhy_t = nc.dram_tensor("enhy_t", [H, BL * S], BF, kind="ExternalInput").ap()
    enhy_n = nc.dram_tensor("enhy_n", [BL * S, H], BF, kind="ExternalInput").ap()
    wen_t = nc.dram_tensor("wen_t", [H, H], BF, kind="ExternalInput").ap()
    wde_t = nc.dram_tensor("wde_t", [H, H], BF, kind="ExternalInput").ap()
    dehy_t = nc.dram_tensor("dehy_t", [H, BL], BF, kind="ExternalInput").ap()
    wwarp = nc.dram_tensor("wwarp", [P, KT], BF, kind="ExternalInput").ap()
    b_en_t = nc.dram_tensor("b_en_t", [P, KT], F32, kind="ExternalInput").ap()
    past = nc.dram_tensor("past", [1, BL * S], F32, kind="ExternalInput").ap()

    out_c = nc.dram_tensor("out_c", [BL, H], F32, kind="ExternalOutput").ap()
    out_attn = nc.dram_tensor("out_attn", [BL, S], F32, kind="ExternalOutput").ap()
    out_ee = nc.dram_tensor("out_ee", [BL, S], F32, kind="ExternalOutput").ap()

    with tile.TileContext(nc) as tc, ExitStack() as ctx:
        consts = ctx.enter_context(tc.tile_pool(name="consts", bufs=1))
        sc_pool = ctx.enter_context(tc.tile_pool(name="ps", bufs=2, space="PSUM"))
        pw_pool = ctx.enter_context(tc.tile_pool(name="pw", bufs=2, space="PSUM"))
        pc_pool = ctx.enter_context(tc.tile_pool(name="pc", bufs=1, space="PSUM"))
        et_pool = ctx.enter_context(tc.tile_pool(name="et", bufs=2))
        enn_pool = ctx.enter_context(tc.tile_pool(name="enn", bufs=8))
        tanh_pool = ctx.enter_context(tc.tile_pool(name="th", bufs=3))
        small_pool = ctx.enter_context(tc.tile_pool(name="small", bufs=3))
        batch_pool = ctx.enter_context(tc.tile_pool(name="batch", bufs=2))

        # ---- constants into SBUF ----
        wen_sb = []
        for ht in range(HT):
            t = consts.tile([P, H], BF, tag=f"wen{ht}")
            nc.sync.dma_start(out=t[:], in_=wen_t[ht * P:(ht + 1) * P, :])
            wen_sb.append(t)
        wwarp_sb = consts.tile([P, KT], BF, tag="wwarp")
        nc.sync.dma_start(out=wwarp_sb[:], in_=wwarp[:, :])
        ben_sb = consts.tile([P, KT], F32, tag="ben")
        nc.sync.dma_start(out=ben_sb[:], in_=b_en_t[:, :])
        past_sb = consts.tile([1, BL * S], F32, tag="past")
        nc.sync.dma_start(out=past_sb[:], in_=past[:, :])
        past_r = consts.tile([1, BL * S], F32, tag="past_r")
        nc.vector.reciprocal(past_r[:], past_sb[:])

        # ---- q = dehy @ W_de.T + b_en, laid out [k partitions, batch] ----
        wde_sb = []
        dehy_sb = []
        for ht in range(HT):
            t = consts.tile([P, H], BF, tag=f"wde{ht}")
            nc.sync.dma_start(out=t[:], in_=wde_t[ht * P:(ht + 1) * P, :])
            wde_sb.append(t)
            d = consts.tile([P, BL], BF, tag=f"dehy{ht}")
            nc.sync.dma_start(out=d[:], in_=dehy_t[ht * P:(ht + 1) * P, :])
            dehy_sb.append(d)
        qb_sb = []
        for kt in range(KT):
            pq = sc_pool.tile([P, BL], F32, tag="ps")
            for ht in range(HT):
                nc.tensor.matmul(pq[:], lhsT=wde_sb[ht][:, kt * P:(kt + 1) * P],
                                 rhs=dehy_sb[ht][:],
                                 start=(ht == 0), stop=(ht == HT - 1))
            qb = consts.tile([P, BL], F32, tag=f"qb{kt}")
            nc.vector.tensor_scalar_add(qb[:], pq[:], ben_sb[:, kt:kt + 1])
            qb_sb.append(qb)

        # ---- main loop over local batches ----
        for b in range(BL):
            w_full = batch_pool.tile([1, S], F32, tag="wfull")
            zpart = batch_pool.tile([1, NCHUNK], F32, tag="zpart")
            pc0 = pc_pool.tile([1, CHUNK], F32, tag="pc0")
            pc1 = pc_pool.tile([1, CHUNK], F32, tag="pc1")
            for c in range(NCHUNK):
                col0 = b * S + c * CHUNK
                ets = []
                for ht in range(HT):
                    t = et_pool.tile([P, CHUNK], BF, tag=f"et{ht}")
                    nc.sync.dma_start(
                        out=t[:], in_=enhy_t[ht * P:(ht + 1) * P, col0:col0 + CHUNK])
                    ets.append(t)
                enns = []
                for st in range(ST):
                    t = enn_pool.tile([P, H], BF, tag="enn")
                    r0 = col0 + st * P
                    nc.sync.dma_start(out=t[:], in_=enhy_n[r0:r0 + P, :])
                    enns.append(t)

                pw = pw_pool.tile([1, CHUNK], F32, tag="pw")
                for kt in range(KT):
                    ps = sc_pool.tile([P, CHUNK], F32, tag="ps")
                    for ht in range(HT):
                        nc.tensor.matmul(
                            ps[:], lhsT=wen_sb[ht][:, kt * P:(kt + 1) * P],
                            rhs=ets[ht][:],
                            start=(ht == 0), stop=(ht == HT - 1))
                    th = tanh_pool.tile([P, CHUNK], BF, tag="th")
                    nc.scalar.activation(th[:], ps[:], AF.Tanh,
                                         bias=qb_sb[kt][:, b:b + 1])
                    nc.tensor.matmul(pw[:], lhsT=wwarp_sb[:, kt:kt + 1],
                                     rhs=th[:],
                                     start=(kt == 0), stop=(kt == KT - 1))

                ee = small_pool.tile([1, CHUNK], F32, tag="ee")
                nc.scalar.activation(ee[:], pw[:], AF.Exp)
                nc.sync.dma_start(out=out_ee[b:b + 1, c * CHUNK:(c + 1) * CHUNK],
                                  in_=ee[:])
                wch = w_full[0:1, c * CHUNK:(c + 1) * CHUNK]
                nc.vector.tensor_mul(wch, ee[:], past_r[0:1, col0:col0 + CHUNK])
                nc.vector.reduce_sum(zpart[0:1, c:c + 1], wch, axis=AX.X)
                wb16 = small_pool.tile([1, CHUNK], BF, tag="wb16")
                nc.vector.tensor_copy(wb16[:], wch)
                w2 = small_pool.tile([P, ST], BF, tag="w2")
                for st in range(ST):
                    nc.sync.dma_start(out=w2[:, st:st + 1],
                                      in_=wb16[0:1, st * P:(st + 1) * P])
                for st in range(ST):
                    g = c * ST + st
                    nc.tensor.matmul(pc0[:], lhsT=w2[:, st:st + 1],
                                     rhs=enns[st][:, 0:CHUNK],
                                     start=(g == 0), stop=(g == NCHUNK * ST - 1))
                    nc.tensor.matmul(pc1[:], lhsT=w2[:, st:st + 1],
                                     rhs=enns[st][:, CHUNK:H],
                                     start=(g == 0), stop=(g == NCHUNK * ST - 1))

            z = small_pool.tile([1, 1], F32, tag="z")
            nc.vector.reduce_sum(z[:], zpart[:], axis=AX.X)
            rz = small_pool.tile([1, 1], F32, tag="rz")
            nc.vector.reciprocal(rz[:], z[:])
            attn_sb = batch_pool.tile([1, S], F32, tag="attn")
            nc.vector.tensor_scalar_mul(attn_sb[:], w_full[:], rz[:])
            nc.sync.dma_start(out=out_attn[b:b + 1, :], in_=attn_sb[:])
            csb = batch_pool.tile([1, H], F32, tag="csb")
            nc.vector.tensor_scalar_mul(csb[0:1, 0:CHUNK], pc0[:], rz[:])
            nc.vector.tensor_scalar_mul(csb[0:1, CHUNK:H], pc1[:], rz[:])
            nc.sync.dma_start(out=out_c[b:b + 1, :], in_=csb[:])

    nc.compile()
    return nc


def _get_nc():
    if "nc" not in _COMPILED:
        _COMPILED["nc"] = _build_nc()
    return _COMPILED["nc"]


def _prep_in_maps(dehy, enhy, past_attn, W_en, b_en, W_de, W_warp):
    bf16 = ml_dtypes.bfloat16
    wen_t = np.ascontiguousarray(W_en.T).astype(bf16)
    wde_t = np.ascontiguousarray(W_de.T).astype(bf16)
    wwarp = np.ascontiguousarray(
        W_warp[0].astype(bf16).reshape(KT, P).T)            # [128, KT]
    b_en_t = np.ascontiguousarray(
        b_en.astype(np.float32).reshape(KT, P).T)           # [128, KT]

    in_maps = []
    for core in range(NCORES):
        b0 = core * BL
        sh = enhy[b0:b0 + BL].astype(bf16)                  # [BL, S, H]
        enhy_n = np.ascontiguousarray(sh.reshape(BL * S, H))
        enhy_t = np.ascontiguousarray(sh.reshape(BL * S, H).T)  # [H, BL*S]
        dehy_t = np.ascontiguousarray(dehy[b0:b0 + BL].astype(bf16).T)
        past = np.ascontiguousarray(
            past_attn[b0:b0 + BL].astype(np.float32).reshape(1, BL * S))
        in_maps.append({
            "enhy_t": enhy_t,
            "enhy_n": enhy_n,
            "wen_t": wen_t,
            "wde_t": wde_t,
            "dehy_t": dehy_t,
            "wwarp": wwarp,
            "b_en_t": b_en_t,
            "past": past,
        })
    return in_maps


def kernel(dehy, enhy, past_attn, W_en, b_en, W_de, W_warp, _trace=False):
    from concourse.bass_utils import run_bass_kernel_spmd

    nc = _get_nc()
    in_maps = _prep_in_maps(dehy, enhy, past_attn, W_en, b_en, W_de, W_warp)
    res = run_bass_kernel_spmd(nc, in_maps, core_ids=list(range(NCORES)),
                               trace=_trace)
    _COMPILED["last_result"] = res

    c_enc = np.concatenate([r["out_c"] for r in res.results], axis=0)
    attn = np.concatenate([r["out_attn"] for r in res.results], axis=0)
    ee = np.concatenate([r["out_ee"] for r in res.results], axis=0)
    return (c_enc.astype(np.float32), attn.astype(np.float32),
            ee.astype(np.float32))
